# revision 4
# baseline (speedup 1.0000x reference)
"""BiLSTM-CRF loss kernel for 8 Trainium2 NeuronCores (optimized).

Sharding: data-parallel over batch (64 -> 8 cores x 8 rows). Each core runs
both LSTM directions for its shard, computes CRF emissions, runs the CRF
forward pass, and writes the partial sum of (forward - gold_emissions) over
its rows. Host sums partials, adds the transition part of the gold score
(host-computable from tags alone) and the constant scan-scale correction.

Device-side structure:
  - All LSTM nonlinearities run on the DVE via one custom 8-stage op
    (clamped odd degree-5 polynomial ~ tanh, max abs err 0.014):
    sigmoid(x) = (tanh(x/2)+1)/2 with the 0.5 input scales folded into
    pre-scaled weights, doubled state (C=2c, H=2h) absorbing output scales.
    The per-step chain is PE matmul block -> 5 DVE ops, no Act engine.
  - Input projection xp (incl. bias via a K=1 ones-matmul) is accumulated
    into the gate PSUM with an identity matmul, so no separate add op.
  - Gate chunk order (i, f, o, g, C) lets one scalar_tensor_tensor compute
    [u|v] = (th[i,f]+1) * [th_g|C] with contiguous slices.
  - The backward direction writes h into original-t slots (reversed), so
    emissions matmuls read both directions with plain strides.
  - Embedding gather / transpose / projection are emitted interleaved with
    the LSTM steps (PE/Act/GpSimd idle slots), only 2 lead-in tiles.
  - CRF forward scan runs in the exp domain with a constant per-step scale
    2^-CSH folded into P = exp(trans) (bf16): A_t = E_t * (P @ A_{t-1}),
    no renormalization at all (validated drift < +-5 log2 around 1.0).
    The scan chain is one matmul + one DVE multiply per step.
"""

import sys

sys.path.insert(0, "/opt/trn_rl_repo")

import numpy as np
import ml_dtypes

import concourse.bass as bass
from concourse import bacc
import concourse.tile as tile
from concourse import mybir
from concourse.bass import IndirectOffsetOnAxis
from concourse.bass_utils import run_bass_kernel_spmd
from concourse.masks import make_identity
import concourse.dve_ops as dve_ops
from concourse.dve_ops import DveOp
from concourse.dve_spec import (
    Spec, Src0, C0, C1, C2, C3, Zero, sq, maxx, minn, _spill_c3_to_src1,
)

F32 = mybir.dt.float32
BF16 = mybir.dt.bfloat16
I32 = mybir.dt.int32
ALU = mybir.AluOpType
AF = mybir.ActivationFunctionType

B, L, E, H, C = 64, 256, 256, 256, 20
G = 4 * H
NCORES = 8
BC = B // NCORES            # batch rows per core
CH = 8                      # gate-hid chunks of 128; c = gate'*2 + half
NT = (L * BC) // 128        # token tiles per direction = 16
TPT = 128 // BC             # timesteps per token tile = 16
START, STOP = 18, 19
CSH = 4.594                 # log2 of folded per-step CRF scale

# clamped odd deg-5 polynomial ~ tanh(x): clip(x*(P0 + P1 x^2 + P2 x^4), +-PA)
P0, P1, P2, PA = 0.94549404, -0.19485749, 0.0195195, 0.98597409

_CACHE = {}


def _register_poly_op():
    """Register the custom DVE op (idempotent). out = clip(in0*(s0 + s1*t +
    imm2*t^2), -clamp, clamp) with t = in0^2, clamp delivered via in1."""
    name = "POLY5_CLAMP_LSTM"
    for op in dve_ops.OPS:
        if op.name == name:
            return op
    t = sq(Src0)
    y = ((C2 * t + C1) * t + C0) * Src0
    body = _spill_c3_to_src1(minn(maxx(y, Zero - C3), C3))
    spec = Spec(
        body=body,
        reference=lambda in0, in1, s0, s1, imm2: np.clip(
            in0 * (s0 + s1 * in0**2 + imm2 * in0**4), -in1, in1
        ).astype(np.float32),
    )
    op = DveOp(name, spec, subdim=False,
               uops_sha={"v3": "a348bada2721ad20", "v4": "346382fbcfb69e1a"})
    dve_ops.OPS.append(op)
    dve_ops._SUB_OPCODE_FOR_NAME[name] = (
        dve_ops._CUSTOM_DVE_ROW_BASE + len(dve_ops.OPS) - 1
    )
    return op


def _build_module():
    poly = _register_poly_op()
    nc = bacc.Bacc(None, target_bir_lowering=False, debug=False)

    # ---- DRAM I/O ----
    d_embed = nc.dram_tensor("embed_bf", [50000, E], BF16, kind="ExternalInput")
    d_idxf = nc.dram_tensor("idx_f", [128, NT], I32, kind="ExternalInput")
    d_idxb = nc.dram_tensor("idx_b", [128, NT], I32, kind="ExternalInput")
    d_wih = nc.dram_tensor("wih", [128, 2, 2, CH, 128], BF16, kind="ExternalInput")
    d_whh = nc.dram_tensor("whh", [128, 2, 2, CH, 128], BF16, kind="ExternalInput")
    d_xbias = nc.dram_tensor("xbias", [1, 2, CH, 128], BF16, kind="ExternalInput")
    d_h0 = nc.dram_tensor("h0T", [128, 2, 2, BC], BF16, kind="ExternalInput")
    d_c0 = nc.dram_tensor("c0T", [128, 2, 2, BC], F32, kind="ExternalInput")
    d_wout = nc.dram_tensor("woutT", [128, 2, 2, C], BF16, kind="ExternalInput")
    d_bout = nc.dram_tensor("bout", [C, 1], F32, kind="ExternalInput")
    d_pplus = nc.dram_tensor("pplusT", [C, C], BF16, kind="ExternalInput")
    d_wstop = nc.dram_tensor("wstop", [C, 1], BF16, kind="ExternalInput")
    d_ohcur = nc.dram_tensor("ohcur", [C, BC, L], F32, kind="ExternalInput")
    d_a0 = nc.dram_tensor("a0", [C, BC], BF16, kind="ExternalInput")
    d_out = nc.dram_tensor("out", [1, 1], F32, kind="ExternalOutput")

    with tile.TileContext(nc) as tc:
        with (
            tc.tile_pool(name="persist", bufs=1) as pp,
            tc.tile_pool(name="work", bufs=3) as wp,
            tc.tile_pool(name="lstm", bufs=3) as lp,
        ):
            # ---- persistent SBUF ----
            wih_sb = pp.tile([128, 2, 2, CH, 128], BF16, tag="wih")
            whh_sb = pp.tile([128, 2, 2, CH, 128], BF16, tag="whh")
            xbias_sb = pp.tile([1, 2, CH, 128], BF16, tag="xbias")
            wout_sb = pp.tile([128, 2, 2, C], BF16, tag="wout")
            bout_sb = pp.tile([C, 1], F32, tag="bout")
            pplus_sb = pp.tile([C, C], BF16, tag="pplus")
            wstop_sb = pp.tile([C, 1], BF16, tag="wstop")
            ohcur_sb = pp.tile([C, BC, L], F32, tag="ohcur")
            idxf_sb = pp.tile([128, NT], I32, tag="idxf")
            idxb_sb = pp.tile([128, NT], I32, tag="idxb")
            ident128 = pp.tile([128, 128], BF16, tag="id128")
            ones_row = pp.tile([1, 128], BF16, tag="ones_row")
            ones1f = pp.tile([1, C], F32, tag="ones1f")
            ones20c = pp.tile([C, 1], F32, tag="ones20c")
            clamp_t = pp.tile([128, 1], F32, tag="clamp")
            # x^T staging: [E-part, k-half, tile, token]
            xT = [pp.tile([128, 2, NT, 128], BF16, name=f"xT{d}", tag=f"xT{d}")
                  for d in (0, 1)]
            # xp: [ghid-part, t, chunk, b]  (bias included)
            xpT = [pp.tile([128, L, CH, BC], BF16, name=f"xpT{d}", tag=f"xpT{d}")
                   for d in (0, 1)]
            # h history: [hid-part, slot(0..L), k-half, b]
            #   dir0: slot s+1 = h after step s (h_f at t = s)
            #   dir1: slot L-1-s = h after step s (h_b at t = L-1-s); init slot L
            hsT = [pp.tile([128, L + 1, 2, BC], BF16, name=f"hsT{d}", tag=f"hsT{d}")
                   for d in (0, 1)]
            # th + C state: chunks 0:8 = tanh(gates) (i,f,o,g), 8:10 = C = 2c
            thc = [pp.tile([128, 10, BC], F32, name=f"thc{d}", tag=f"thc{d}")
                   for d in (0, 1)]
            featsT = pp.tile([C, L * BC], F32, tag="featsT")
            eT = pp.tile([C, L * BC], F32, tag="eT")
            mrow = pp.tile([1, L * BC], F32, tag="mrow")
            fm = pp.tile([C, L * BC], F32, tag="fm")
            prod = pp.tile([C, BC, L], F32, tag="prod")
            gsum2 = pp.tile([C, BC], F32, tag="gsum2")
            msum = pp.tile([1, BC], F32, tag="msum")
            avec = pp.tile([C, 2, BC], BF16, tag="avec")
            flog = pp.tile([1, BC], F32, tag="flog")
            emit_s = pp.tile([1, BC], F32, tag="emit_s")
            fsum = pp.tile([1, BC], F32, tag="fsum")
            lp_t = pp.tile([1, 1], F32, tag="lp")

            # ---- load constants ----
            nc.sync.dma_start(out=wih_sb[:], in_=d_wih[:])
            nc.sync.dma_start(out=whh_sb[:], in_=d_whh[:])
            nc.sync.dma_start(out=xbias_sb[:], in_=d_xbias[:])
            nc.sync.dma_start(out=wout_sb[:], in_=d_wout[:])
            nc.sync.dma_start(out=bout_sb[:], in_=d_bout[:])
            nc.sync.dma_start(out=pplus_sb[:], in_=d_pplus[:])
            nc.sync.dma_start(out=wstop_sb[:], in_=d_wstop[:])
            nc.sync.dma_start(out=ohcur_sb[:], in_=d_ohcur[:])
            nc.sync.dma_start(out=idxf_sb[:], in_=d_idxf[:])
            nc.sync.dma_start(out=idxb_sb[:], in_=d_idxb[:])
            # initial states: dir0 h at slot 0, dir1 h at slot L; C into thc
            nc.sync.dma_start(out=hsT[0][:, 0, :, :], in_=d_h0[:, 0, :, :])
            nc.sync.dma_start(out=hsT[1][:, L, :, :], in_=d_h0[:, 1, :, :])
            for d in (0, 1):
                nc.sync.dma_start(out=thc[d][:, 8:10, :], in_=d_c0[:, d, :, :])
            nc.sync.dma_start(out=avec[:, 0, :], in_=d_a0[:])
            make_identity(nc, ident128[:])
            nc.vector.memset(ones_row[:], 1.0)
            nc.vector.memset(ones1f[:], 1.0)
            nc.vector.memset(ones20c[:], 1.0)
            nc.vector.memset(clamp_t[:], PA)

            # ---- P1 pipeline: gather -> transpose -> project (+bias) ----
            ps_p1 = tc.tile_pool(name="ps_p1", bufs=1, space="PSUM")
            ps1 = ps_p1.__enter__()
            ps_tr = tc.tile_pool(name="ps_tr", bufs=2, space="PSUM")
            pst = ps_tr.__enter__()

            def p1_groups(d, ti):
                idx_sb = idxf_sb if d == 0 else idxb_sb
                gx = wp.tile([128, E], BF16, tag=f"gx{d}")

                def g_gather():
                    nc.gpsimd.indirect_dma_start(
                        out=gx[:], out_offset=None, in_=d_embed[:],
                        in_offset=IndirectOffsetOnAxis(
                            ap=idx_sb[:, ti:ti + 1], axis=0),
                    )

                def g_tr(k):
                    def go():
                        pt = pst.tile([128, 128], BF16, tag="ptr")
                        nc.tensor.transpose(
                            pt[:], gx[:, k * 128:(k + 1) * 128], ident128[:])
                        nc.scalar.activation(
                            xT[d][:, k, ti, :], pt[:], AF.Copy)
                    return go

                pj = ps1.tile([128, CH, 128], F32, tag="pj")

                def g_proj(c0i, c1i):
                    def go():
                        for cc in range(c0i, c1i):
                            nc.tensor.matmul(
                                pj[:, cc, :], wih_sb[:, d, 0, cc, :],
                                xT[d][:, 0, ti, :], start=True, stop=False)
                            nc.tensor.matmul(
                                pj[:, cc, :], wih_sb[:, d, 1, cc, :],
                                xT[d][:, 1, ti, :], start=False, stop=False)
                            nc.tensor.matmul(
                                pj[:, cc, :], xbias_sb[0:1, d, cc, :],
                                ones_row[0:1, :], start=False, stop=True)
                    return go

                def g_move():
                    # pj [p, c, (tl b)] -> xpT [p, t=ti*16+tl, c, b]
                    nc.scalar.activation(
                        xpT[d][:, ti * TPT:(ti + 1) * TPT, :, :],
                        pj[:].rearrange("p c (tl b) -> p tl c b", b=BC),
                        AF.Copy,
                    )

                return [g_gather, g_tr(0), g_tr(1),
                        g_proj(0, 4), g_proj(4, 8), g_move]

            # lead-in: tiles 0,1 for both dirs
            backlog = []
            for ti in range(NT):
                for d in (0, 1):
                    gs = p1_groups(d, ti)
                    if ti < 2:
                        for g in gs:
                            g()
                    else:
                        backlog.extend(gs)
            backlog.reverse()  # pop() from the front order

            # ---- P2: LSTM recurrence, both directions phase-interleaved ----
            ps_p2 = tc.tile_pool(name="ps_p2", bufs=2, space="PSUM")
            ps2 = ps_p2.__enter__()
            h0c, h1c = (P0 * 0.5, P1 * 0.125)   # tanh(x/2) coeffs
            h2c = P2 * 0.03125
            for s in range(L):
                for d in (0, 1):
                    sl_in = s if d == 0 else L - s
                    sl_out = s + 1 if d == 0 else L - 1 - s
                    pg = ps2.tile([128, CH, BC], F32, tag=f"pg{d}")
                    for c in range(CH):
                        nc.tensor.matmul(
                            pg[:, c, :], whh_sb[:, d, 0, c, :],
                            hsT[d][:, sl_in, 0, :], start=True, stop=False)
                        nc.tensor.matmul(
                            pg[:, c, :], whh_sb[:, d, 1, c, :],
                            hsT[d][:, sl_in, 1, :], start=False, stop=False)
                    nc.tensor.matmul(
                        pg[:].rearrange("p c b -> p (c b)"), ident128[:],
                        xpT[d][:, s, :, :].rearrange("p c b -> p (c b)"),
                        start=False, stop=True, skip_group_check=True)
                    # th = poly_tanh(pg)  (gates prescaled: i,f,o by 0.5)
                    nc.vector._custom_dve(
                        poly, out=thc[d][:, 0:8, :], in0=pg[:],
                        in1=clamp_t[:], s0=P0, s1=P1, imm2=P2)
                    # [u|v] = (th[i,f] + 1) * [th_g | C]
                    uv = lp.tile([128, 4, BC], F32, tag=f"uv{d}")
                    nc.vector.scalar_tensor_tensor(
                        out=uv[:], in0=thc[d][:, 0:4, :], scalar=1.0,
                        in1=thc[d][:, 6:10, :], op0=ALU.add, op1=ALU.mult)
                    # C' = v*0.5 + u
                    nc.vector.scalar_tensor_tensor(
                        out=thc[d][:, 8:10, :], in0=uv[:, 2:4, :], scalar=0.5,
                        in1=uv[:, 0:2, :], op0=ALU.mult, op1=ALU.add)
                    # tcc = poly_tanh(C'/2)  (coeffs absorb the halving)
                    tcc = lp.tile([128, 2, BC], F32, tag=f"tcc{d}")
                    nc.vector._custom_dve(
                        poly, out=tcc[:], in0=thc[d][:, 8:10, :],
                        in1=clamp_t[:], s0=h0c, s1=h1c, imm2=h2c)
                    # H = (th_o + 1) * tcc   (= 2 h, folded into weights)
                    nc.vector.scalar_tensor_tensor(
                        out=hsT[d][:, sl_out, :, :], in0=thc[d][:, 4:6, :],
                        scalar=1.0, in1=tcc[:], op0=ALU.add, op1=ALU.mult)
                # trickle P1 work into idle engine slots
                for _ in range(2):
                    if backlog:
                        backlog.pop()()
            while backlog:
                backlog.pop()()
            ps_p2.__exit__(None, None, None)
            ps_tr.__exit__(None, None, None)
            ps_p1.__exit__(None, None, None)

            # ---- P3: emissions feats^T[C, (t b)] = sum_d Wout_d @ H_d + b ----
            ps_p3 = tc.tile_pool(name="ps_p3", bufs=1, space="PSUM")
            ps3 = ps_p3.__enter__()
            pf = ps3.tile([C, L * BC], F32, tag="big")
            for n in range(4):
                seg = slice(n * 512, (n + 1) * 512)
                tseg0 = slice(1 + n * 64, 1 + (n + 1) * 64)  # dir0: slot t+1
                tseg1 = slice(n * 64, (n + 1) * 64)          # dir1: slot t
                for k in (0, 1):
                    nc.tensor.matmul(
                        pf[:, seg], wout_sb[:, 0, k, :],
                        hsT[0][:, tseg0, k, :], start=(k == 0), stop=False)
                    nc.tensor.matmul(
                        pf[:, seg], wout_sb[:, 1, k, :],
                        hsT[1][:, tseg1, k, :], start=False, stop=(k == 1))
            nc.scalar.activation(featsT[:], pf[:], AF.Identity, bias=bout_sb[:])

            # per-(t,b) max over tags (partition reduce on GpSimd)
            nc.gpsimd.tensor_reduce(
                out=mrow[:], in_=featsT[:], axis=mybir.AxisListType.C,
                op=ALU.max)
            # msum[b] = sum_t m[t,b]
            nc.vector.tensor_reduce(
                out=msum[:],
                in_=mrow[:].rearrange("a (t b) -> a b t", b=BC),
                axis=mybir.AxisListType.X, op=ALU.add)
            # broadcast m to all tag partitions, subtract, exp
            pm = ps3.tile([C, L * BC], F32, tag="big")
            for n in range(4):
                seg = slice(n * 512, (n + 1) * 512)
                nc.tensor.matmul(pm[:, seg], ones1f[:], mrow[0:1, seg],
                                 start=True, stop=True)
            nc.vector.scalar_tensor_tensor(
                out=fm[:], in0=featsT[:], scalar=0.0, in1=pm[:],
                op0=ALU.add, op1=ALU.subtract)
            nc.scalar.activation(eT[:], fm[:], AF.Exp)

            # gold emissions: sum_t feats[tag_t]  via one-hot dot
            nc.vector.scalar_tensor_tensor(
                out=prod[:],
                in0=featsT[:].rearrange("p (t b) -> p b t", b=BC),
                scalar=0.0, in1=ohcur_sb[:],
                op0=ALU.add, op1=ALU.mult)
            nc.vector.tensor_reduce(
                out=gsum2[:], in_=prod[:], axis=mybir.AxisListType.X,
                op=ALU.add)
            pemit = ps3.tile([1, BC], F32, tag="pemit")
            nc.tensor.matmul(pemit[:], ones20c[:], gsum2[:],
                             start=True, stop=True)
            nc.vector.tensor_copy(emit_s[:], pemit[:])
            ps_p3.__exit__(None, None, None)

            # ---- P5: CRF forward scan, A_t = E_t * (P @ A_{t-1}) ----
            ps_p5 = tc.tile_pool(name="ps_p5", bufs=4, space="PSUM")
            ps5 = ps_p5.__enter__()
            for t in range(L):
                pa = ps5.tile([C, BC], F32, tag="pa")
                nc.tensor.matmul(
                    pa[:], pplus_sb[:], avec[:, t % 2, :], start=True,
                    stop=True)
                nc.vector.scalar_tensor_tensor(
                    out=avec[:, (t + 1) % 2, :], in0=pa[:], scalar=1.0,
                    in1=eT[:, t * BC:(t + 1) * BC], op0=ALU.mult,
                    op1=ALU.mult)

            # ---- P6: finalize ----
            paf = ps5.tile([1, BC], F32, tag="paf", bufs=1)
            nc.tensor.matmul(paf[:], wstop_sb[:], avec[:, L % 2, :],
                             start=True, stop=True)
            nc.scalar.activation(flog[:], paf[:], AF.Ln)
            nc.vector.scalar_tensor_tensor(
                out=fsum[:], in0=flog[:], scalar=0.0, in1=msum[:],
                op0=ALU.add, op1=ALU.add)
            nc.vector.scalar_tensor_tensor(
                out=fsum[:], in0=fsum[:], scalar=0.0, in1=emit_s[:],
                op0=ALU.add, op1=ALU.subtract)
            nc.vector.tensor_reduce(
                out=lp_t[:], in_=fsum[:], axis=mybir.AxisListType.X,
                op=ALU.add)
            nc.sync.dma_start(out=d_out[:], in_=lp_t[:])
            ps_p5.__exit__(None, None, None)

    nc.finalize()
    return nc


def _prep_inmaps(inputs):
    bf = ml_dtypes.bfloat16
    sent = np.asarray(inputs["sentences"])
    tags = np.asarray(inputs["tags"])
    embed = np.asarray(inputs["embed"], dtype=np.float32)
    trans = np.asarray(inputs["transitions"], dtype=np.float32)
    h0 = np.asarray(inputs["h0"], dtype=np.float32)
    c0 = np.asarray(inputs["c0"], dtype=np.float32)
    W_out = np.asarray(inputs["W_out"], dtype=np.float32)
    b_out = np.asarray(inputs["b_out"], dtype=np.float32)

    embed_bf = np.ascontiguousarray(embed.astype(bf))

    # gate reorder PyTorch (i,f,g,o) -> chunk order (i,f,o,g); scale i,f,o
    # rows by 0.5 (tanh-half trick); Whh by extra 0.5 (doubled h).
    gmap = [0, 1, 3, 2]
    rs = np.array([0.5, 0.5, 0.5, 1.0], np.float32)  # in (i,f,o,g) order

    def chunk_weights(W, extra):
        # W [G, Kin] (PyTorch row order) -> [128, Kin//128, CH, 128]
        Kin = W.shape[1]
        Wr = W.reshape(4, 2, 128, Kin // 128, 128)  # [gate,half,m,k,p]
        Wr = Wr[gmap] * (rs * extra)[:, None, None, None, None]
        return np.ascontiguousarray(
            Wr.transpose(4, 3, 0, 1, 2).reshape(128, Kin // 128, CH, 128))

    wih = np.zeros((128, 2, 2, CH, 128), np.float32)
    whh = np.zeros((128, 2, 2, CH, 128), np.float32)
    xbias = np.zeros((1, 2, CH, 128), np.float32)
    for d, (Wih, Whh, b) in enumerate(
        [(inputs["Wih_f"], inputs["Whh_f"], inputs["b_f"]),
         (inputs["Wih_b"], inputs["Whh_b"], inputs["b_b"])]
    ):
        wih[:, d] = chunk_weights(np.asarray(Wih, np.float32), 1.0)
        whh[:, d] = chunk_weights(np.asarray(Whh, np.float32), 0.5)
        bt = np.asarray(b, np.float32).reshape(4, 2, 128)[gmap] * rs[:, None, None]
        xbias[0, d] = bt.reshape(CH, 128)
    wih = np.ascontiguousarray(wih.astype(bf))
    whh = np.ascontiguousarray(whh.astype(bf))
    xbias = np.ascontiguousarray(xbias.astype(bf))

    # wout^T [p, d, k, m] = 0.5 * W_out[m, d*256 + k*128 + p]
    wout = np.ascontiguousarray(
        (0.5 * W_out).reshape(C, 2, 2, 128).transpose(3, 1, 2, 0).astype(bf))
    bout = np.ascontiguousarray(b_out[:, None].astype(np.float32))
    pplus = np.ascontiguousarray(
        (np.exp(trans) * 2.0 ** -CSH).T.astype(bf))
    wstop = np.ascontiguousarray(np.exp(trans[STOP, :])[:, None].astype(bf))

    # host part of the gold score: transition sum (tags are inputs)
    te = np.concatenate(
        [np.full((B, 1), START, tags.dtype), tags], axis=1)
    gold_trans = trans[te[:, 1:], te[:, :-1]].sum(1) + trans[STOP, te[:, -1]]
    host_const = CSH * np.log(2.0) * L - gold_trans.mean()

    ar = np.arange(C)
    in_maps = []
    for q in range(NCORES):
        bs = slice(q * BC, (q + 1) * BC)
        sq_ = sent[bs]
        tq = tags[bs]
        idx_f = np.ascontiguousarray(
            sq_.T.reshape(NT, TPT, BC).transpose(1, 2, 0)
            .reshape(128, NT).astype(np.int32))
        sqr = sq_[:, ::-1]
        idx_b = np.ascontiguousarray(
            sqr.T.reshape(NT, TPT, BC).transpose(1, 2, 0)
            .reshape(128, NT).astype(np.int32))
        h0q = np.ascontiguousarray(
            (2.0 * h0[:, bs, :]).reshape(2, BC, 2, 128)
            .transpose(3, 0, 2, 1).astype(bf))
        c0q = np.ascontiguousarray(
            (2.0 * c0[:, bs, :]).reshape(2, BC, 2, 128)
            .transpose(3, 0, 2, 1).astype(np.float32))
        ohcur = (ar[:, None, None] == tq[None, :, :]).astype(np.float32)
        a0 = ((ar[:, None] == START) * np.ones((1, BC))).astype(bf)
        in_maps.append({
            "embed_bf": embed_bf, "idx_f": idx_f, "idx_b": idx_b,
            "wih": wih, "whh": whh, "xbias": xbias,
            "h0T": h0q, "c0T": c0q, "woutT": wout, "bout": bout,
            "pplusT": pplus, "wstop": wstop,
            "ohcur": np.ascontiguousarray(ohcur),
            "a0": np.ascontiguousarray(a0),
        })
    return in_maps, host_const


def get_module():
    if "nc" not in _CACHE:
        _CACHE["nc"] = _build_module()
    return _CACHE["nc"]


def kernel(**inputs):
    nc = get_module()
    in_maps, host_const = _prep_inmaps(inputs)
    res = run_bass_kernel_spmd(nc, in_maps, core_ids=list(range(NCORES)))
    total = sum(float(r["out"][0, 0]) for r in res.results)
    return np.float32(total / B + host_const)


# revision 11
# speedup vs baseline: 1.4736x; 1.4736x over previous
"""BiLSTM-CRF loss kernel for 8 Trainium2 NeuronCores (optimized).

Sharding: data-parallel over batch (64 -> 8 cores x 8 rows). Each core runs
both LSTM directions for its shard, computes CRF emissions, runs the CRF
forward pass, and writes the partial sum of (forward - gold_emissions) over
its rows. Host sums partials, adds the transition part of the gold score
(host-computable from tags alone) and the constant scan-scale correction.

Device-side structure:
  - All LSTM nonlinearities run on the DVE via one custom 8-stage op
    (clamped odd degree-5 polynomial ~ tanh, max abs err 0.014):
    sigmoid(x) = (tanh(x/2)+1)/2 with the 0.5 input scales folded into
    pre-scaled weights, doubled state (C=2c, H=2h) absorbing output scales.
    The per-step chain is PE matmul block -> 5 DVE ops, no Act engine.
  - Input projection xp (incl. bias via a K=1 ones-matmul) is accumulated
    into the gate PSUM with an identity matmul, so no separate add op.
  - Gate chunk order (i, f, o, g, C) lets one scalar_tensor_tensor compute
    [u|v] = (th[i,f]+1) * [th_g|C] with contiguous slices.
  - The backward direction writes h into original-t slots (reversed), so
    emissions matmuls read both directions with plain strides.
  - Embedding gather / transpose / projection are emitted interleaved with
    the LSTM steps (PE/Act/GpSimd idle slots), only 2 lead-in tiles.
  - CRF forward scan runs in the exp domain with a constant per-step scale
    2^-CSH folded into P = exp(trans) (bf16): A_t = E_t * (P @ A_{t-1}),
    no renormalization at all (validated drift < +-5 log2 around 1.0).
    The scan chain is one matmul + one DVE multiply per step.
"""

import sys

sys.path.insert(0, "/opt/trn_rl_repo")

import numpy as np
import ml_dtypes

import concourse.bass as bass
from concourse import bacc
import concourse.tile as tile
from concourse import mybir
from concourse.bass import IndirectOffsetOnAxis
from concourse.bass_utils import run_bass_kernel_spmd
from concourse.masks import make_identity
import concourse.dve_ops as dve_ops
from concourse.dve_ops import DveOp
from concourse.dve_spec import (
    Spec, Src0, C0, C1, C2, C3, Zero, sq, maxx, minn, _spill_c3_to_src1,
)

F32 = mybir.dt.float32
BF16 = mybir.dt.bfloat16
I32 = mybir.dt.int32
ALU = mybir.AluOpType
AF = mybir.ActivationFunctionType

B, L, E, H, C = 64, 256, 256, 256, 20
G = 4 * H
NCORES = 8
BC = B // NCORES            # batch rows per core
CH = 8                      # gate-hid chunks of 128; c = gate'*2 + half
NT = (L * BC) // 128        # token tiles per direction = 16
TPT = 128 // BC             # timesteps per token tile = 16
START, STOP = 18, 19
CSH = 4.594                 # log2 of folded per-step CRF scale

# clamped odd deg-5 polynomial ~ tanh(x): clip(x*(P0 + P1 x^2 + P2 x^4), +-PA)
P0, P1, P2, PA = 0.94549404, -0.19485749, 0.0195195, 0.98597409

_CACHE = {}


def _register_poly_op():
    """Register the custom DVE op (idempotent). out = clip(in0*(s0 + s1*t +
    imm2*t^2), -clamp, clamp) with t = in0^2, clamp delivered via in1."""
    name = "POLY5_CLAMP_LSTM"
    for op in dve_ops.OPS:
        if op.name == name:
            return op
    t = sq(Src0)
    y = ((C2 * t + C1) * t + C0) * Src0
    body = _spill_c3_to_src1(minn(maxx(y, Zero - C3), C3))
    spec = Spec(
        body=body,
        reference=lambda in0, in1, s0, s1, imm2: np.clip(
            in0 * (s0 + s1 * in0**2 + imm2 * in0**4), -in1, in1
        ).astype(np.float32),
    )
    op = DveOp(name, spec, subdim=False,
               uops_sha={"v3": "a348bada2721ad20", "v4": "346382fbcfb69e1a"})
    dve_ops.OPS.append(op)
    dve_ops._SUB_OPCODE_FOR_NAME[name] = (
        dve_ops._CUSTOM_DVE_ROW_BASE + len(dve_ops.OPS) - 1
    )
    return op


def _build_module():
    poly = _register_poly_op()
    nc = bacc.Bacc(None, target_bir_lowering=False, debug=False)

    # ---- DRAM I/O ----
    d_embed = nc.dram_tensor("embed_bf", [50000, E], BF16, kind="ExternalInput")
    d_idxf = nc.dram_tensor("idx_f", [128, NT], I32, kind="ExternalInput")
    d_idxb = nc.dram_tensor("idx_b", [128, NT], I32, kind="ExternalInput")
    d_wih = nc.dram_tensor("wih", [128, 2, 2, CH, 128], BF16, kind="ExternalInput")
    d_whh = nc.dram_tensor("whh", [128, 2, 2, CH, 128], BF16, kind="ExternalInput")
    d_xbiasT = nc.dram_tensor("xbiasT", [CH, 2, 128], BF16, kind="ExternalInput")
    d_ind = nc.dram_tensor("indCN", [CH, CH, 256], BF16, kind="ExternalInput")
    d_h0 = nc.dram_tensor("h0T", [128, 2, 2, BC], BF16, kind="ExternalInput")
    d_c0 = nc.dram_tensor("c0T", [128, 2, 2, BC], F32, kind="ExternalInput")
    d_wout = nc.dram_tensor("woutT", [128, 2, 2, C], BF16, kind="ExternalInput")
    d_bout = nc.dram_tensor("bout", [C, 1], F32, kind="ExternalInput")
    d_pplus = nc.dram_tensor("pplusT", [C, C], BF16, kind="ExternalInput")
    d_wstop = nc.dram_tensor("wstop", [C, 1], BF16, kind="ExternalInput")
    d_ohcur = nc.dram_tensor("ohcur", [C, BC, L], F32, kind="ExternalInput")
    d_a0 = nc.dram_tensor("a0", [C, BC], BF16, kind="ExternalInput")
    d_out = nc.dram_tensor("out", [1, 1], F32, kind="ExternalOutput")

    with tile.TileContext(nc) as tc:
        with (
            tc.tile_pool(name="persist", bufs=1) as pp,
            tc.tile_pool(name="work", bufs=3) as wp,
            tc.tile_pool(name="lstm", bufs=3) as lp,
        ):
            # ---- persistent SBUF ----
            wih_sb = pp.tile([128, 2, 2, CH, 128], BF16, tag="wih")
            whh_sb = pp.tile([128, 2, 2, CH, 128], BF16, tag="whh")
            xbiasT_sb = pp.tile([CH, 2, 128], BF16, tag="xbiasT")
            ind_sb = pp.tile([CH, CH, 256], BF16, tag="indCN")
            ident20 = pp.tile([C, C], F32, tag="id20")
            s_tiles = pp.tile([128, NT], F32, tag="stiles")
            s_row = pp.tile([1, L * BC], F32, tag="srow")
            wout_sb = pp.tile([128, 2, 2, C], BF16, tag="wout")
            bout_sb = pp.tile([C, 1], F32, tag="bout")
            pplus_sb = pp.tile([C, C], BF16, tag="pplus")
            wstop_sb = pp.tile([C, 1], BF16, tag="wstop")
            ohcur_sb = pp.tile([C, BC, L], F32, tag="ohcur")
            idxf_sb = pp.tile([128, NT], I32, tag="idxf")
            idxb_sb = pp.tile([128, NT], I32, tag="idxb")
            ident128 = pp.tile([128, 128], BF16, tag="id128")
            ones_row = pp.tile([1, 128], BF16, tag="ones_row")
            ones1f = pp.tile([1, C], F32, tag="ones1f")
            ones20c = pp.tile([C, 1], F32, tag="ones20c")
            clamp_t = pp.tile([128, 1], F32, tag="clamp")
            # x^T staging: [E-part, k-half, tile, token]
            xT = [pp.tile([128, 2, NT, 128], BF16, name=f"xT{d}", tag=f"xT{d}")
                  for d in (0, 1)]
            # xp: [ghid-part, t, chunk, b]  (bias included)
            xpT = [pp.tile([128, L, CH, BC], BF16, name=f"xpT{d}", tag=f"xpT{d}")
                   for d in (0, 1)]
            # h history: [hid-part, slot(0..L), k-half, b]
            #   dir0: slot s+1 = h after step s (h_f at t = s)
            #   dir1: slot L-1-s = h after step s (h_b at t = L-1-s); init slot L
            hsT = [pp.tile([128, L + 1, 2, BC], BF16, name=f"hsT{d}", tag=f"hsT{d}")
                   for d in (0, 1)]
            # th + C state: chunks 0:8 = tanh(gates) (i,f,o,g), 8:10 = C = 2c
            thc = [pp.tile([128, 10, BC], F32, name=f"thc{d}", tag=f"thc{d}")
                   for d in (0, 1)]
            featsT = pp.tile([128, L * BC], F32, tag="featsT")
            eT = pp.tile([C, L * BC], F32, tag="eT")

            fm = pp.tile([C, L * BC], F32, tag="fm")
            prod = pp.tile([C, BC, L], F32, tag="prod")
            gsum2 = pp.tile([C, BC], F32, tag="gsum2")
            msum = pp.tile([1, BC], F32, tag="msum")
            avec = pp.tile([C, 2, BC], BF16, tag="avec")
            flog = pp.tile([1, BC], F32, tag="flog")
            emit_s = pp.tile([1, BC], F32, tag="emit_s")
            fsum = pp.tile([1, BC], F32, tag="fsum")
            lp_t = pp.tile([1, 1], F32, tag="lp")

            # ---- load constants ----
            nc.sync.dma_start(out=wih_sb[:], in_=d_wih[:])
            nc.sync.dma_start(out=whh_sb[:], in_=d_whh[:])
            nc.sync.dma_start(out=xbiasT_sb[:], in_=d_xbiasT[:])
            nc.sync.dma_start(out=ind_sb[:], in_=d_ind[:])
            nc.sync.dma_start(out=wout_sb[:], in_=d_wout[:])
            nc.sync.dma_start(out=bout_sb[:], in_=d_bout[:])
            nc.sync.dma_start(out=pplus_sb[:], in_=d_pplus[:])
            nc.sync.dma_start(out=wstop_sb[:], in_=d_wstop[:])
            nc.sync.dma_start(out=ohcur_sb[:], in_=d_ohcur[:])
            nc.sync.dma_start(out=idxf_sb[:], in_=d_idxf[:])
            nc.sync.dma_start(out=idxb_sb[:], in_=d_idxb[:])
            # initial states: dir0 h at slot 0, dir1 h at slot L; C into thc
            nc.sync.dma_start(out=hsT[0][:, 0, :, :], in_=d_h0[:, 0, :, :])
            nc.sync.dma_start(out=hsT[1][:, L, :, :], in_=d_h0[:, 1, :, :])
            for d in (0, 1):
                nc.sync.dma_start(out=thc[d][:, 8:10, :], in_=d_c0[:, d, :, :])
            nc.sync.dma_start(out=avec[:, 0, :], in_=d_a0[:])
            make_identity(nc, ident128[:])
            make_identity(nc, ident20[:])
            nc.vector.memset(ones_row[:], 1.0)
            nc.vector.memset(ones1f[:], 1.0)
            nc.vector.memset(ones20c[:], 1.0)
            nc.vector.memset(clamp_t[:], PA)

            # ---- P1 pipeline: gather -> transpose -> project (+bias) ----
            ps_p1 = tc.tile_pool(name="ps_p1", bufs=1, space="PSUM")
            ps1 = ps_p1.__enter__()
            ps_tr = tc.tile_pool(name="ps_tr", bufs=2, space="PSUM")
            pst = ps_tr.__enter__()

            def p1_groups(d, tp):
                # tile pair tp covers token tiles 2*tp, 2*tp+1
                idx_sb = idxf_sb if d == 0 else idxb_sb
                gx = [wp.tile([128, E], BF16, name=f"gx{d}_{i}", tag=f"gx{d}_{i}")
                      for i in (0, 1)]

                def g_gather(i):
                    def go():
                        nc.gpsimd.indirect_dma_start(
                            out=gx[i][:], out_offset=None, in_=d_embed[:],
                            in_offset=IndirectOffsetOnAxis(
                                ap=idx_sb[:, 2 * tp + i:2 * tp + i + 1], axis=0),
                        )
                    return go

                def g_tr(i, k):
                    def go():
                        pt = pst.tile([128, 128], BF16, tag="ptr")
                        nc.tensor.transpose(
                            pt[:], gx[i][:, k * 128:(k + 1) * 128], ident128[:])
                        nc.scalar.activation(
                            xT[d][:, k, 2 * tp + i, :], pt[:], AF.Copy)
                    return go

                pj = ps1.tile([128, CH, 256], F32, tag="pj")

                def g_proj(cc):
                    def go():
                        nc.tensor.matmul(
                            pj[:, cc, :], wih_sb[:, d, 0, cc, :],
                            xT[d][:, 0, 2 * tp:2 * tp + 2, :].rearrange(
                                "p i n -> p (i n)"),
                            start=True, stop=False)
                        nc.tensor.matmul(
                            pj[:, cc, :], wih_sb[:, d, 1, cc, :],
                            xT[d][:, 1, 2 * tp:2 * tp + 2, :].rearrange(
                                "p i n -> p (i n)"),
                            start=False, stop=False)
                    return go

                def g_bias():
                    # pj[p, c, n] += xbias[p, c]  (indicator-matrix matmul,
                    # split to respect the 512-col PSUM-bank limit)
                    pjf = pj[:].rearrange("p c n -> p (c n)")
                    indf = ind_sb[:].rearrange("p c n -> p (c n)")
                    for q4 in range(4):
                        sl = slice(q4 * 512, (q4 + 1) * 512)
                        nc.tensor.matmul(
                            pjf[:, sl], xbiasT_sb[:, d, :], indf[:, sl],
                            start=False, stop=(q4 == 3),
                            skip_group_check=True)

                def g_move():
                    # pj [p, c, (i tl b)] -> xpT [p, t=tp*32+(i tl), c, b]
                    nc.scalar.activation(
                        xpT[d][:, tp * 32:(tp + 1) * 32, :, :],
                        pj[:].rearrange("p c (tt b) -> p tt c b", b=BC),
                        AF.Copy,
                    )

                return ([g_gather(0), g_gather(1),
                         g_tr(0, 0), g_tr(0, 1), g_tr(1, 0), g_tr(1, 1)]
                        + [g_proj(cc) for cc in range(CH)]
                        + [g_bias, g_move])

            # lead-in: tile pairs 0,1 (token tiles 0..3) for both dirs
            backlog = []
            for tp in range(NT // 2):
                for d in (0, 1):
                    gs = p1_groups(d, tp)
                    if tp < 2:
                        for g in gs:
                            g()
                    else:
                        backlog.extend(gs)
            backlog.reverse()  # pop() from the front order

            # ---- P2: LSTM recurrence, both directions phase-interleaved ----
            ps_p2 = tc.tile_pool(name="ps_p2", bufs=2, space="PSUM")
            ps2 = ps_p2.__enter__()
            h0c, h1c = (P0 * 0.5, P1 * 0.125)   # tanh(x/2) coeffs
            h2c = P2 * 0.03125
            for s in range(L):
                pgs = ps2.tile([128, 2, CH, BC], F32, tag="pg")
                for d in (0, 1):
                    sl_in = s if d == 0 else L - s
                    sl_out = s + 1 if d == 0 else L - 1 - s
                    pg = pgs[:, d]
                    for c in range(CH):
                        nc.tensor.matmul(
                            pg[:, c, :], whh_sb[:, d, 0, c, :],
                            hsT[d][:, sl_in, 0, :], start=True, stop=False)
                        nc.tensor.matmul(
                            pg[:, c, :], whh_sb[:, d, 1, c, :],
                            hsT[d][:, sl_in, 1, :], start=False, stop=False)
                    nc.tensor.matmul(
                        pg[:].rearrange("p c b -> p (c b)"), ident128[:],
                        xpT[d][:, s, :, :].rearrange("p c b -> p (c b)"),
                        start=False, stop=True, skip_group_check=True)
                    # th = poly_tanh(pg)  (gates prescaled: i,f,o by 0.5)
                    nc.vector._custom_dve(
                        poly, out=thc[d][:, 0:8, :], in0=pg[:],
                        in1=clamp_t[:], s0=P0, s1=P1, imm2=P2)
                    # [u|v] = (th[i,f] + 1) * [th_g | C]
                    uv = lp.tile([128, 4, BC], F32, tag=f"uv{d}")
                    nc.vector.scalar_tensor_tensor(
                        out=uv[:], in0=thc[d][:, 0:4, :], scalar=1.0,
                        in1=thc[d][:, 6:10, :], op0=ALU.add, op1=ALU.mult)
                    # C' = v*0.5 + u
                    nc.vector.scalar_tensor_tensor(
                        out=thc[d][:, 8:10, :], in0=uv[:, 2:4, :], scalar=0.5,
                        in1=uv[:, 0:2, :], op0=ALU.mult, op1=ALU.add)
                    # tcc = poly_tanh(C'/2)  (coeffs absorb the halving)
                    tcc = lp.tile([128, 2, BC], F32, tag=f"tcc{d}")
                    nc.vector._custom_dve(
                        poly, out=tcc[:], in0=thc[d][:, 8:10, :],
                        in1=clamp_t[:], s0=h0c, s1=h1c, imm2=h2c)
                    # H = (th_o + 1) * tcc   (= 2 h, folded into weights)
                    nc.vector.scalar_tensor_tensor(
                        out=hsT[d][:, sl_out, :, :], in0=thc[d][:, 4:6, :],
                        scalar=1.0, in1=tcc[:], op0=ALU.add, op1=ALU.mult)
                    # trickle P1 work into idle engine slots
                    if backlog:
                        backlog.pop()()
            while backlog:
                backlog.pop()()
            ps_p2.__exit__(None, None, None)
            ps_tr.__exit__(None, None, None)
            ps_p1.__exit__(None, None, None)

            # ---- P3: emissions feats^T[C, (t b)] = sum_d Wout_d @ H_d + b ----
            ps_p3 = tc.tile_pool(name="ps_p3", bufs=1, space="PSUM")
            ps3 = ps_p3.__enter__()
            pf = ps3.tile([C, L * BC], F32, tag="big")
            for n in range(4):
                seg = slice(n * 512, (n + 1) * 512)
                tseg0 = slice(1 + n * 64, 1 + (n + 1) * 64)  # dir0: slot t+1
                tseg1 = slice(n * 64, (n + 1) * 64)          # dir1: slot t
                for k in (0, 1):
                    nc.tensor.matmul(
                        pf[:, seg], wout_sb[:, 0, k, :],
                        hsT[0][:, tseg0, k, :], start=(k == 0), stop=False)
                    nc.tensor.matmul(
                        pf[:, seg], wout_sb[:, 1, k, :],
                        hsT[1][:, tseg1, k, :], start=False, stop=(k == 1))
            nc.vector.memset(featsT[:], -3.0e38)
            nc.scalar.activation(
                featsT[0:C, :], pf[:], AF.Identity, bias=bout_sb[:])

            # per-(t,b) max over tags: PE transpose + free-dim reduce
            fv = featsT[:].rearrange("p (ti n) -> p ti n", ti=NT)
            for ti in range(NT):
                ptf = ps3.tile([128, C], F32, tag="ptf", bufs=2)
                nc.tensor.transpose(ptf[:], fv[0:C, ti, :], ident20[:])
                nc.vector.tensor_reduce(
                    out=s_tiles[:, ti:ti + 1], in_=ptf[:],
                    axis=mybir.AxisListType.X, op=ALU.max)
            # s_row[0, p*NT + ti] = s_tiles[p, ti]
            nc.sync.dma_start(out=s_row[0:1, :], in_=s_tiles[:])
            # msum[b] = sum_t m[t,b]
            nc.vector.tensor_reduce(
                out=msum[:],
                in_=s_row[0:1, :].rearrange(
                    "a (tl b ti) -> a b tl ti", b=BC, ti=NT),
                axis=mybir.AxisListType.XY, op=ALU.add)
            # broadcast m over tag partitions; reorder (tl,b,ti)->(ti,tl,b)
            sv = s_row[0:1, :].rearrange(
                "a (tl b ti) -> a ti tl b", b=BC, ti=NT)
            pm = ps3.tile([C, L * BC], F32, tag="big")
            for n in range(4):
                seg = slice(n * 512, (n + 1) * 512)
                nc.tensor.matmul(pm[:, seg], ones1f[:],
                                 sv[:, n * 4:(n + 1) * 4, :, :],
                                 start=True, stop=True)
            nc.vector.scalar_tensor_tensor(
                out=fm[:], in0=featsT[0:C, :], scalar=0.0, in1=pm[:],
                op0=ALU.add, op1=ALU.subtract)
            nc.scalar.activation(eT[:], fm[:], AF.Exp)

            # gold emissions: sum_t feats[tag_t]  via one-hot dot
            nc.vector.scalar_tensor_tensor(
                out=prod[:],
                in0=featsT[0:C, :].rearrange("p (t b) -> p b t", b=BC),
                scalar=0.0, in1=ohcur_sb[:],
                op0=ALU.add, op1=ALU.mult)
            nc.vector.tensor_reduce(
                out=gsum2[:], in_=prod[:], axis=mybir.AxisListType.X,
                op=ALU.add)
            pemit = ps3.tile([1, BC], F32, tag="pemit")
            nc.tensor.matmul(pemit[:], ones20c[:], gsum2[:],
                             start=True, stop=True)
            nc.vector.tensor_copy(emit_s[:], pemit[:])
            ps_p3.__exit__(None, None, None)

            # ---- P5: CRF forward scan, A_t = E_t * (P @ A_{t-1}) ----
            ps_p5 = tc.tile_pool(name="ps_p5", bufs=4, space="PSUM")
            ps5 = ps_p5.__enter__()
            for t in range(L):
                pa = ps5.tile([C, BC], F32, tag="pa")
                nc.tensor.matmul(
                    pa[:], pplus_sb[:], avec[:, t % 2, :], start=True,
                    stop=True)
                nc.vector.scalar_tensor_tensor(
                    out=avec[:, (t + 1) % 2, :], in0=pa[:], scalar=1.0,
                    in1=eT[:, t * BC:(t + 1) * BC], op0=ALU.mult,
                    op1=ALU.mult)

            # ---- P6: finalize ----
            paf = ps5.tile([1, BC], F32, tag="paf", bufs=1)
            nc.tensor.matmul(paf[:], wstop_sb[:], avec[:, L % 2, :],
                             start=True, stop=True)
            nc.scalar.activation(flog[:], paf[:], AF.Ln)
            nc.vector.scalar_tensor_tensor(
                out=fsum[:], in0=flog[:], scalar=0.0, in1=msum[:],
                op0=ALU.add, op1=ALU.add)
            nc.vector.scalar_tensor_tensor(
                out=fsum[:], in0=fsum[:], scalar=0.0, in1=emit_s[:],
                op0=ALU.add, op1=ALU.subtract)
            nc.vector.tensor_reduce(
                out=lp_t[:], in_=fsum[:], axis=mybir.AxisListType.X,
                op=ALU.add)
            nc.sync.dma_start(out=d_out[:], in_=lp_t[:])
            ps_p5.__exit__(None, None, None)

    nc.finalize()
    return nc


def _prep_inmaps(inputs):
    bf = ml_dtypes.bfloat16
    sent = np.asarray(inputs["sentences"])
    tags = np.asarray(inputs["tags"])
    embed = np.asarray(inputs["embed"], dtype=np.float32)
    trans = np.asarray(inputs["transitions"], dtype=np.float32)
    h0 = np.asarray(inputs["h0"], dtype=np.float32)
    c0 = np.asarray(inputs["c0"], dtype=np.float32)
    W_out = np.asarray(inputs["W_out"], dtype=np.float32)
    b_out = np.asarray(inputs["b_out"], dtype=np.float32)

    embed_bf = np.ascontiguousarray(embed.astype(bf))

    # gate reorder PyTorch (i,f,g,o) -> chunk order (i,f,o,g); scale i,f,o
    # rows by 0.5 (tanh-half trick); Whh by extra 0.5 (doubled h).
    gmap = [0, 1, 3, 2]
    rs = np.array([0.5, 0.5, 0.5, 1.0], np.float32)  # in (i,f,o,g) order

    def chunk_weights(W, extra):
        # W [G, Kin] (PyTorch row order) -> [128, Kin//128, CH, 128]
        Kin = W.shape[1]
        Wr = W.reshape(4, 2, 128, Kin // 128, 128)  # [gate,half,m,k,p]
        Wr = Wr[gmap] * (rs * extra)[:, None, None, None, None]
        return np.ascontiguousarray(
            Wr.transpose(4, 3, 0, 1, 2).reshape(128, Kin // 128, CH, 128))

    wih = np.zeros((128, 2, 2, CH, 128), np.float32)
    whh = np.zeros((128, 2, 2, CH, 128), np.float32)
    xbiasT = np.zeros((CH, 2, 128), np.float32)
    for d, (Wih, Whh, b) in enumerate(
        [(inputs["Wih_f"], inputs["Whh_f"], inputs["b_f"]),
         (inputs["Wih_b"], inputs["Whh_b"], inputs["b_b"])]
    ):
        wih[:, d] = chunk_weights(np.asarray(Wih, np.float32), 1.0)
        whh[:, d] = chunk_weights(np.asarray(Whh, np.float32), 0.5)
        bt = np.asarray(b, np.float32).reshape(4, 2, 128)[gmap] * rs[:, None, None]
        xbiasT[:, d] = bt.reshape(CH, 128)
    wih = np.ascontiguousarray(wih.astype(bf))
    whh = np.ascontiguousarray(whh.astype(bf))
    xbiasT = np.ascontiguousarray(xbiasT.astype(bf))
    indCN = np.ascontiguousarray(
        (np.arange(CH)[:, None, None] == np.arange(CH)[None, :, None])
        * np.ones((1, 1, 256)), dtype=None).astype(bf)

    # wout^T [p, d, k, m] = 0.5 * W_out[m, d*256 + k*128 + p]
    wout = np.ascontiguousarray(
        (0.5 * W_out).reshape(C, 2, 2, 128).transpose(3, 1, 2, 0).astype(bf))
    bout = np.ascontiguousarray(b_out[:, None].astype(np.float32))
    pplus = np.ascontiguousarray(
        (np.exp(trans) * 2.0 ** -CSH).T.astype(bf))
    wstop = np.ascontiguousarray(np.exp(trans[STOP, :])[:, None].astype(bf))

    # host part of the gold score: transition sum (tags are inputs)
    te = np.concatenate(
        [np.full((B, 1), START, tags.dtype), tags], axis=1)
    gold_trans = trans[te[:, 1:], te[:, :-1]].sum(1) + trans[STOP, te[:, -1]]
    host_const = CSH * np.log(2.0) * L - gold_trans.mean()

    ar = np.arange(C)
    in_maps = []
    for q in range(NCORES):
        bs = slice(q * BC, (q + 1) * BC)
        sq_ = sent[bs]
        tq = tags[bs]
        idx_f = np.ascontiguousarray(
            sq_.T.reshape(NT, TPT, BC).transpose(1, 2, 0)
            .reshape(128, NT).astype(np.int32))
        sqr = sq_[:, ::-1]
        idx_b = np.ascontiguousarray(
            sqr.T.reshape(NT, TPT, BC).transpose(1, 2, 0)
            .reshape(128, NT).astype(np.int32))
        h0q = np.ascontiguousarray(
            (2.0 * h0[:, bs, :]).reshape(2, BC, 2, 128)
            .transpose(3, 0, 2, 1).astype(bf))
        c0q = np.ascontiguousarray(
            (2.0 * c0[:, bs, :]).reshape(2, BC, 2, 128)
            .transpose(3, 0, 2, 1).astype(np.float32))
        ohcur = (ar[:, None, None] == tq[None, :, :]).astype(np.float32)
        a0 = ((ar[:, None] == START) * np.ones((1, BC))).astype(bf)
        in_maps.append({
            "embed_bf": embed_bf, "idx_f": idx_f, "idx_b": idx_b,
            "wih": wih, "whh": whh, "xbiasT": xbiasT, "indCN": indCN,
            "h0T": h0q, "c0T": c0q, "woutT": wout, "bout": bout,
            "pplusT": pplus, "wstop": wstop,
            "ohcur": np.ascontiguousarray(ohcur),
            "a0": np.ascontiguousarray(a0),
        })
    return in_maps, host_const


def get_module():
    if "nc" not in _CACHE:
        _CACHE["nc"] = _build_module()
    return _CACHE["nc"]


def kernel(**inputs):
    nc = get_module()
    in_maps, host_const = _prep_inmaps(inputs)
    res = run_bass_kernel_spmd(nc, in_maps, core_ids=list(range(NCORES)))
    total = sum(float(r["out"][0, 0]) for r in res.results)
    return np.float32(total / B + host_const)
